# revision 52
# baseline (speedup 1.0000x reference)
"""AttentionDCA pseudo-likelihood loss on 8 Trainium2 NeuronCores.

Data-parallel over the MSA axis M (1024 sequences per core).

Host (cheap): attention map A, RBF kernel Vaa, coupling matrix
Jmat[(r,q),(j,a)] = sum_h A[h,r,j] Vaa[h,q,a] with the r==j diagonal
blocks zeroed.  Jmat is symmetric (A and Vaa are both symmetric), so the
same buffer serves as the matmul rhs without a transpose.

Device (dominant): E^T[m, f] = sum_k Zoh[k, m] * Jmat[k, f] as fp8-E4M3
matmuls in DoubleRow perf mode (K=256 per instruction), f tiled in
chunks of 504 = 24 complete 21-wide q-groups.  The epilogue is fused on
device: per chunk, exp -> 21-group sums -> Ln (with row accumulation)
gives sum_r log-sum-exp_q E, and scalar_tensor_tensor against the
transposed one-hot gives sum_r E[Z[r,m]].  Only 2 partial scalars per
(m, chunk) leave the device (90 KB/core instead of 22 MB).

The graph is raw Bass blocks (explicit per-engine streams + semaphores):
the TileContext scheduler emits multi-wait instructions this container's
walrus build rejects ("Too many sync wait commands").

J is pre-scaled by SCALE (folded into Vaa); the Exp activation's scale
argument un-scales it, and the host divides the Ec partials by SCALE.
E >= 0 and max E ~ 4 here, so the logsumexp needs no max-shift (guarded
by an upper bound computed on the host; falls back to CPU if violated).
"""

import sys
import numpy as np

for p in ("/opt/trn_rl_repo", "/root/.axon_site/_ro/trn_rl_repo"):
    if p not in sys.path:
        sys.path.insert(0, p)

import ml_dtypes

import concourse.bass as bass
from concourse import mybir
from concourse.bass_utils import run_bass_kernel_spmd

Q_AA = 21
H = 32
L = 256
DK = 32
M_TOT = 8192
N_CORES = 8
M_LOC = M_TOT // N_CORES          # 1024
MB = M_LOC // 128                 # 8 output-row blocks (m)
F = L * Q_AA                      # 5376 flattened (pos, aa) dim
KB = F // 128                     # 42 K-blocks of 128
KP = KB // 2                      # 21 DoubleRow pairs
CHUNKS = [504] * 10 + [336]           # f-chunks, each a whole number of
                                      # 21-wide q-groups (psum bank caps at 504)
NCH = len(CHUNKS)
NT = NCH * MB                     # 88 matmul chains
LAMBDA = 1e-3
SCALE = 256.0                     # J pre-scale for fp8 range use
FP8 = ml_dtypes.float8_e4m3fn
BF16 = ml_dtypes.bfloat16

NPSUM = 6                         # psum banks in rotation
NE = 3                            # exp-result buffers

_CACHE = {}
LAST_RESULTS = None               # for test harness introspection
DEVICE_NS = None                  # wall-clock of the device submit+run+fetch


def _build_graph():
    if "nc" in _CACHE:
        return _CACHE["nc"]
    nc = bass.Bass()
    f8 = mybir.dt.float8e4
    bf = mybir.dt.bfloat16
    f32 = mybir.dt.float32
    # jt / zoht are chunk-major pre-tiled on the host so every chunk DMA is
    # one contiguous run per partition (sub-512B segments cost 2x DMA time);
    # zoh is mb-major so PE can start after the first 1/8th arrives
    jt_ext = nc.declare_dram_parameter("jt", [128, KB * F], f8, isOutput=False)
    zoh_ext = nc.declare_dram_parameter("zoh", [128, MB, KB, 128], f8, isOutput=False)
    zoht_ext = nc.declare_dram_parameter("zoht", [128, MB * F], f8, isOutput=False)
    out_ext = nc.declare_dram_parameter("out", [128, NT * 2], f32, isOutput=True)

    from contextlib import ExitStack
    with ExitStack() as ctx:
        def sem(name):
            return ctx.enter_context(nc.semaphore(name))

        def sb(name, shape, dtype):
            return ctx.enter_context(nc.sbuf_tensor(name, shape, dtype))

        s_dz0 = sem("s_dz0")      # zoh loaded
        s_dj = sem("s_dj")        # jt chunks loaded (SP queue)
        s_dj2 = sem("s_dj2")      # jt chunk-0 first half (ACT queue)
        s_dz = sem("s_dz")        # zht chunks loaded
        s_mm = sem("s_mm")        # matmul chains done
        s_exp = sem("s_exp")      # exp done
        s_ln = sem("s_ln")        # ln done
        s_red = sem("s_red")      # group-sum done
        s_ec = sem("s_ec")        # ec stt done
        s_out = sem("s_out")      # partials stored (Pool queue)

        zoh = sb("sb_zoh", [128, MB, KB, 128], f8)
        # flat chunk buffers: every chunk lands contiguous regardless of size
        jtb = [sb(f"jtb{i}", [128, KB * 504], f8) for i in range(2)]
        zhb = [sb(f"zhb{i}", [128, MB * 504], f8) for i in range(2)]
        et = [sb(f"et{i}", [128, 504], f32) for i in range(NE)]
        se = [sb(f"se{i}", [128, 24], f32) for i in range(2)]
        lse = sb("lse", [128, 24], f32)
        scr = sb("scr", [128, 504], f32)
        pt = sb("pt", [128, NT * 2], f32)
        acc = [
            ctx.enter_context(nc.psum_tensor(f"acc{i}", [128, 504], f32))
            for i in range(NPSUM)
        ]

        with nc.Block() as block:

            HALF0 = (KB // 2) * CHUNKS[0]   # chunk-0 split point (kb 0..20)

            @block.sync
            def _(sync):
                # SP queue: jt chunk stream (chunk 0 second half only — the
                # first half arrives in parallel on the ACT queue so PE can
                # start ~4us earlier)
                c0 = 0
                for c, NC in enumerate(CHUNKS):
                    if c >= 2:
                        # buffer c%2 readers: all chains of chunk c-2
                        sync.wait_ge(s_mm, 8 * (c - 1))
                    lo = KB * c0 + (HALF0 if c == 0 else 0)
                    sync.dma_start(
                        out=jtb[c % 2][:, (HALF0 if c == 0 else 0):KB * NC],
                        in_=jt_ext[:, lo:KB * (c0 + NC)],
                    ).then_inc(s_dj, 16)
                    c0 += NC

            @block.gpsimd
            def _(gp):
                # Pool queue: one-hots in, partials out — overlaps the SP jt
                # stream.  zht c0 is needed by the DVE Ec path right after
                # the first chain, so it goes out after just two zoh blocks.
                for mb in range(2):
                    gp.dma_start(
                        out=zoh[:, mb], in_=zoh_ext[:, mb]
                    ).then_inc(s_dz0, 16)
                gp.dma_start(
                    out=zhb[0][:, 0:MB * CHUNKS[0]],
                    in_=zoht_ext[:, 0:MB * CHUNKS[0]],
                ).then_inc(s_dz, 16)
                for mb in range(2, MB):
                    gp.dma_start(
                        out=zoh[:, mb], in_=zoh_ext[:, mb]
                    ).then_inc(s_dz0, 16)
                c0 = CHUNKS[0]
                for c, NC in enumerate(CHUNKS):
                    if c == 0:
                        continue
                    if c >= 2:
                        gp.wait_ge(s_ec, 8 * (c - 1))
                    gp.dma_start(
                        out=zhb[c % 2][:, 0:MB * NC],
                        in_=zoht_ext[:, MB * c0:MB * (c0 + NC)],
                    ).then_inc(s_dz, 16)
                    c0 += NC
                gp.wait_ge(s_ln, NT)
                gp.wait_ge(s_ec, NT)
                gp.dma_start(out=out_ext[:], in_=pt[:]).then_inc(s_out, 16)
                gp.wait_ge(s_out, 16)

            @block.tensor
            def _(tensor):
                # warmup: ramp the PE clock to full p-state during the input
                # DMA window (results are garbage, bank is reset by the first
                # real start=True chain; PE is in-order so no sync needed)
                tensor.wait_ge(s_dz0, 16)
                zw = zoh[:, 0].rearrange("p k m -> p (k m)")
                for w in range(9):
                    tensor.matmul(
                        acc[NPSUM - 1][:, 0:504],
                        zw[:, 0:256].rearrange("p (t m) -> p t m", t=2),
                        zw[:, 0:1008].rearrange("p (t n) -> p t n", t=2),
                        start=True,
                        stop=True,
                        perf_mode=mybir.MatmulPerfMode.DoubleRow,
                    )
                for c, NC in enumerate(CHUNKS):
                    for mb in range(MB):
                        t = c * MB + mb
                        if c == 0:
                            tensor.wait_ge(s_dz0, 16 * (mb + 1))
                            if mb == 0:
                                tensor.wait_ge(s_dj2, 16)
                        if mb == 0:
                            tensor.wait_ge(s_dj, 16 * (c + 1))
                        if t >= NPSUM and t % 3 == 0:
                            # psum banks free for chains t..t+2: exp + ec of
                            # chain t-NPSUM+2 done (covers all three)
                            tensor.wait_ge(s_exp, t - NPSUM + 3)
                            tensor.wait_ge(s_ec, t - NPSUM + 3)
                        a = acc[t % NPSUM]
                        for kk in range(KP):
                            ins = tensor.matmul(
                                a[:, 0:NC],
                                zoh[:, mb, 2 * kk:2 * kk + 2, :],
                                jtb[c % 2][
                                    :, 2 * kk * NC:(2 * kk + 2) * NC
                                ].rearrange("p (t n) -> p t n", t=2),
                                start=(kk == 0),
                                stop=(kk == KP - 1),
                                perf_mode=mybir.MatmulPerfMode.DoubleRow,
                            )
                        ins.then_inc(s_mm)

            @block.scalar
            def _(scalar):
                # chunk-0 first half, in parallel with SP's second half
                scalar.dma_start(
                    out=jtb[0][:, 0:HALF0], in_=jt_ext[:, 0:HALF0]
                ).then_inc(s_dj2, 16)
                for c, NC in enumerate(CHUNKS):
                    G = NC // Q_AA
                    for mb in range(MB):
                        t = c * MB + mb
                        scalar.wait_ge(s_mm, t + 1)
                        if t >= NE:
                            scalar.wait_ge(s_red, t - NE + 1)
                        scalar.activation(
                            et[t % NE][:, 0:NC],
                            acc[t % NPSUM][:, 0:NC],
                            mybir.ActivationFunctionType.Exp,
                            scale=1.0 / SCALE,
                        ).then_inc(s_exp)
                        scalar.wait_ge(s_red, t + 1)
                        scalar.activation(
                            lse[:, 0:G],
                            se[t % 2][:, 0:G],
                            mybir.ActivationFunctionType.Ln,
                            accum_out=pt[:, 2 * t:2 * t + 1],
                        ).then_inc(s_ln)

            @block.vector
            def _(vector):
                for c, NC in enumerate(CHUNKS):
                    G = NC // Q_AA
                    for mb in range(MB):
                        t = c * MB + mb
                        vector.wait_ge(s_exp, t + 1)
                        if t >= 2:
                            vector.wait_ge(s_ln, t - 1)
                        vector.tensor_reduce(
                            se[t % 2][:, 0:G],
                            et[t % NE][:, 0:NC].rearrange(
                                "p (g q) -> p g q", q=Q_AA
                            ),
                            axis=mybir.AxisListType.X,
                            op=mybir.AluOpType.add,
                        ).then_inc(s_red)
                        if mb == 0:
                            vector.wait_ge(s_dz, 16 * (c + 1))
                        vector.scalar_tensor_tensor(
                            out=scr[:, 0:NC],
                            in0=acc[t % NPSUM][:, 0:NC],
                            scalar=1.0,
                            in1=zhb[c % 2][:, mb * NC:(mb + 1) * NC],
                            op0=mybir.AluOpType.mult,
                            op1=mybir.AluOpType.mult,
                            accum_out=pt[:, 2 * t + 1:2 * t + 2],
                        ).then_inc(s_ec)

    _CACHE["nc"] = nc
    return nc


def _softmax(x, axis):
    x = x - x.max(axis=axis, keepdims=True)
    e = np.exp(x)
    return e / e.sum(axis=axis, keepdims=True)


def _prologue(reps_matrix, Q, K, V_metric):
    """A, Vaa, and the scaled coupling tensor X4[i,j,q,a]; plus reg term."""
    scores = np.matmul(Q, K.transpose(0, 2, 1)) / np.sqrt(np.float32(DK))
    probs = _softmax(scores, -1)
    A = 0.5 * (probs + probs.transpose(0, 2, 1))           # (H, L, L)

    V1 = np.einsum("qd,hdv->hqv", reps_matrix, V_metric)   # (H, q, dv)
    gamma = 1.0 / V1.shape[1]
    sq = np.sum(V1 * V1, axis=-1)
    D2 = sq[:, :, None] + sq[:, None, :] - 2.0 * np.einsum("hqv,hav->hqa", V1, V1)
    Vaa = np.exp(-gamma * np.maximum(D2, 0.0))             # (H, q, q)

    # X4[i,j,q,a] = SCALE * sum_h A[h,i,j] Vaa[h,q,a], diagonal i==j zeroed
    X = A.reshape(H, L * L).T @ (Vaa * SCALE).reshape(H, Q_AA * Q_AA)
    X4 = X.reshape(L, L, Q_AA, Q_AA)
    X4[np.arange(L), np.arange(L)] = 0.0
    Xf = X4.reshape(-1)
    reg = LAMBDA * float(np.dot(Xf, Xf)) / (SCALE * SCALE)
    return X4, reg


def kernel(reps_matrix, Q, K, V_metric, Z, weights):
    global LAST_RESULTS, DEVICE_NS
    reps_matrix = np.asarray(reps_matrix, np.float32)
    Q = np.asarray(Q, np.float32)
    K = np.asarray(K, np.float32)
    V_metric = np.asarray(V_metric, np.float32)
    Zi = np.asarray(Z).astype(np.int64)
    weights = np.asarray(weights, np.float32)

    X4, reg = _prologue(reps_matrix, Q, K, V_metric)

    # Safety bound for the shift-free on-device logsumexp:
    # max_{q,r,m} E <= max_{q,r} sum_j max_a J[r,j,q,a]
    emax = float(X4.max(axis=3).sum(axis=1).max()) / SCALE
    colidx = np.arange(L)[:, None] * Q_AA + Zi             # (L, M)

    s_all = None
    if emax < 80.0:
        try:
            # Jmat[(i,q),(j,a)] is symmetric; device K-layout [kp, kb, f]
            Jmat = X4.transpose(0, 2, 1, 3).reshape(F, F)
            jt8 = Jmat.reshape(KB, 128, F).astype(FP8)
            jt_np = jt8.transpose(1, 0, 2)          # (128, KB, F) view
            # chunk-major pre-tile: per partition, chunk c is one
            # contiguous (KB*Nc) block
            bounds = np.cumsum([0] + CHUNKS)
            jt_np = np.concatenate(
                [np.ascontiguousarray(jt_np[:, :, a:b]).reshape(128, -1)
                 for a, b in zip(bounds[:-1], bounds[1:])], axis=1
            )

            in_maps = []
            for c in range(N_CORES):
                ci = colidx[:, c * M_LOC:(c + 1) * M_LOC]
                zfull = np.zeros((F, M_LOC), FP8)
                zfull[ci, np.arange(M_LOC)[None, :]] = 1.0
                # [kp, mb, kb, mcol] so each mb-block is one contiguous DMA
                zoh_np = np.ascontiguousarray(
                    zfull.reshape(KB, 128, MB, 128).transpose(1, 2, 0, 3)
                )
                zt = np.zeros((M_LOC, F), FP8)
                zt[np.arange(M_LOC)[:, None], ci.T] = 1.0
                zoht_np = zt.reshape(MB, 128, F).transpose(1, 0, 2)
                zoht_np = np.concatenate(
                    [np.ascontiguousarray(zoht_np[:, :, a:b]).reshape(128, -1)
                     for a, b in zip(bounds[:-1], bounds[1:])], axis=1
                )
                in_maps.append({"jt": jt_np, "zoh": zoh_np, "zoht": zoht_np})

            nc = _build_graph()
            import time as _time
            _t0 = _time.time()
            res = run_bass_kernel_spmd(nc, in_maps, list(range(N_CORES)))
            DEVICE_NS = int((_time.time() - _t0) * 1e9)
            LAST_RESULTS = res
            parts = []
            for c in range(N_CORES):
                o = np.asarray(res.results[c]["out"], np.float32)
                o = o.reshape(128, NCH, MB, 2)        # slot = (c*MB+mb)*2+k
                sl = o[..., 0].sum(axis=1)            # (128, MB) sum_r lse
                ec = o[..., 1].sum(axis=1) / SCALE    # (128, MB) sum_r Ec
                parts.append((ec - sl).T.reshape(M_LOC))  # m = mb*128+p
            s_all = np.concatenate(parts)             # (M,)
        except Exception:
            import traceback
            print("kernel: device path failed, CPU fallback:", file=sys.stderr)
            traceback.print_exc()
            s_all = None

    if s_all is None:
        # host fallback: same contraction in fp32 on CPU
        Jmat = X4.transpose(0, 2, 1, 3).reshape(F, F) / SCALE
        parts = []
        for c in range(N_CORES):
            ci = colidx[:, c * M_LOC:(c + 1) * M_LOC]
            zfull = np.zeros((F, M_LOC), np.float32)
            zfull[ci, np.arange(M_LOC)[None, :]] = 1.0
            E = (Jmat @ zfull).reshape(L, Q_AA, M_LOC)
            mx = E.max(axis=1)
            lge = mx + np.log(np.sum(np.exp(E - mx[:, None, :]), axis=1))
            loc = Zi[:, c * M_LOC:(c + 1) * M_LOC]
            Ec = np.take_along_axis(E, loc[:, None, :], axis=1)[:, 0, :]
            parts.append(np.sum(Ec - lge, axis=0))
        s_all = np.concatenate(parts)

    pl = -float(np.dot(weights, s_all))
    return np.float32(pl + reg)


# revision 53
# speedup vs baseline: 1.0058x; 1.0058x over previous
"""AttentionDCA pseudo-likelihood loss on 8 Trainium2 NeuronCores.

Data-parallel over the MSA axis M (1024 sequences per core).

Host (cheap): attention map A, RBF kernel Vaa, coupling matrix
Jmat[(r,q),(j,a)] = sum_h A[h,r,j] Vaa[h,q,a] with the r==j diagonal
blocks zeroed.  Jmat is symmetric (A and Vaa are both symmetric), so the
same buffer serves as the matmul rhs without a transpose.

Device (dominant): E^T[m, f] = sum_k Zoh[k, m] * Jmat[k, f] as fp8-E4M3
matmuls in DoubleRow perf mode (K=256 per instruction), f tiled in
chunks of 504 = 24 complete 21-wide q-groups.  The epilogue is fused on
device: per chunk, exp -> 21-group sums -> Ln (with row accumulation)
gives sum_r log-sum-exp_q E, and scalar_tensor_tensor against the
transposed one-hot gives sum_r E[Z[r,m]].  Only 2 partial scalars per
(m, chunk) leave the device (90 KB/core instead of 22 MB).

The graph is raw Bass blocks (explicit per-engine streams + semaphores):
the TileContext scheduler emits multi-wait instructions this container's
walrus build rejects ("Too many sync wait commands").

J is pre-scaled by SCALE (folded into Vaa); the Exp activation's scale
argument un-scales it, and the host divides the Ec partials by SCALE.
E >= 0 and max E ~ 4 here, so the logsumexp needs no max-shift (guarded
by an upper bound computed on the host; falls back to CPU if violated).
"""

import sys
import numpy as np

for p in ("/opt/trn_rl_repo", "/root/.axon_site/_ro/trn_rl_repo"):
    if p not in sys.path:
        sys.path.insert(0, p)

import ml_dtypes

import concourse.bass as bass
from concourse import mybir
from concourse.bass_utils import run_bass_kernel_spmd

Q_AA = 21
H = 32
L = 256
DK = 32
M_TOT = 8192
N_CORES = 8
M_LOC = M_TOT // N_CORES          # 1024
MB = M_LOC // 128                 # 8 output-row blocks (m)
F = L * Q_AA                      # 5376 flattened (pos, aa) dim
KB = F // 128                     # 42 K-blocks of 128
KP = KB // 2                      # 21 DoubleRow pairs
CHUNKS = [504] * 10 + [336]           # f-chunks, each a whole number of
                                      # 21-wide q-groups (psum bank caps at 504)
NCH = len(CHUNKS)
NT = NCH * MB                     # 88 matmul chains
LAMBDA = 1e-3
SCALE = 256.0                     # J pre-scale for fp8 range use
FP8 = ml_dtypes.float8_e4m3fn
BF16 = ml_dtypes.bfloat16

NPSUM = 6                         # psum banks in rotation
NE = 3                            # exp-result buffers

_CACHE = {}
LAST_RESULTS = None               # for test harness introspection
DEVICE_NS = None                  # wall-clock of the device submit+run+fetch


def _build_graph():
    if "nc" in _CACHE:
        return _CACHE["nc"]
    nc = bass.Bass()
    f8 = mybir.dt.float8e4
    bf = mybir.dt.bfloat16
    f32 = mybir.dt.float32
    # jt / zoht are chunk-major pre-tiled on the host so every chunk DMA is
    # one contiguous run per partition (sub-512B segments cost 2x DMA time);
    # zoh is mb-major so PE can start after the first 1/8th arrives
    jt_ext = nc.declare_dram_parameter("jt", [128, KB * F], f8, isOutput=False)
    zoh_ext = nc.declare_dram_parameter("zoh", [128, MB, KB, 128], f8, isOutput=False)
    zoht_ext = nc.declare_dram_parameter("zoht", [128, MB * F], f8, isOutput=False)
    out_ext = nc.declare_dram_parameter("out", [128, NT * 2], f32, isOutput=True)
    # raw E block of the final chain — its lse/Ec run on the host so the
    # device tail is a Copy + DMA on idle queues instead of Exp->reduce->Ln
    oute_ext = nc.declare_dram_parameter(
        "oute", [128, CHUNKS[-1]], f32, isOutput=True
    )

    from contextlib import ExitStack
    with ExitStack() as ctx:
        def sem(name):
            return ctx.enter_context(nc.semaphore(name))

        def sb(name, shape, dtype):
            return ctx.enter_context(nc.sbuf_tensor(name, shape, dtype))

        s_dz0 = sem("s_dz0")      # zoh loaded
        s_dj = sem("s_dj")        # jt chunks loaded (SP queue)
        s_dj2 = sem("s_dj2")      # jt chunk-0 first half (ACT queue)
        s_dz = sem("s_dz")        # zht chunks loaded
        s_mm = sem("s_mm")        # matmul chains done
        s_exp = sem("s_exp")      # exp done
        s_ln = sem("s_ln")        # ln done
        s_red = sem("s_red")      # group-sum done
        s_ec = sem("s_ec")        # ec stt done
        s_cp = sem("s_cp")        # final-chain E copied to sbuf
        s_out = sem("s_out")      # partials stored (Pool queue)
        s_oute = sem("s_oute")    # raw E block stored (SP queue)

        zoh = sb("sb_zoh", [128, MB, KB, 128], f8)
        # flat chunk buffers: every chunk lands contiguous regardless of size
        jtb = [sb(f"jtb{i}", [128, KB * 504], f8) for i in range(2)]
        zhb = [sb(f"zhb{i}", [128, MB * 504], f8) for i in range(2)]
        et = [sb(f"et{i}", [128, 504], f32) for i in range(NE)]
        se = [sb(f"se{i}", [128, 24], f32) for i in range(2)]
        lse = sb("lse", [128, 24], f32)
        scr = sb("scr", [128, 504], f32)
        eL = sb("eL", [128, CHUNKS[-1]], f32)
        pt = sb("pt", [128, NT * 2], f32)
        acc = [
            ctx.enter_context(nc.psum_tensor(f"acc{i}", [128, 504], f32))
            for i in range(NPSUM)
        ]

        with nc.Block() as block:

            HALF0 = (KB // 2) * CHUNKS[0]   # chunk-0 split point (kb 0..20)

            @block.sync
            def _(sync):
                # SP queue: jt chunk stream (chunk 0 second half only — the
                # first half arrives in parallel on the ACT queue so PE can
                # start ~4us earlier)
                c0 = 0
                for c, NC in enumerate(CHUNKS):
                    if c >= 2:
                        # buffer c%2 readers: all chains of chunk c-2
                        sync.wait_ge(s_mm, 8 * (c - 1))
                    lo = KB * c0 + (HALF0 if c == 0 else 0)
                    sync.dma_start(
                        out=jtb[c % 2][:, (HALF0 if c == 0 else 0):KB * NC],
                        in_=jt_ext[:, lo:KB * (c0 + NC)],
                    ).then_inc(s_dj, 16)
                    c0 += NC
                sync.wait_ge(s_cp, 1)
                sync.dma_start(out=oute_ext[:], in_=eL[:]).then_inc(s_oute, 16)
                sync.wait_ge(s_oute, 16)

            @block.gpsimd
            def _(gp):
                # Pool queue: one-hots in, partials out — overlaps the SP jt
                # stream.  zht c0 is needed by the DVE Ec path right after
                # the first chain, so it goes out after just two zoh blocks.
                for mb in range(2):
                    gp.dma_start(
                        out=zoh[:, mb], in_=zoh_ext[:, mb]
                    ).then_inc(s_dz0, 16)
                gp.dma_start(
                    out=zhb[0][:, 0:MB * CHUNKS[0]],
                    in_=zoht_ext[:, 0:MB * CHUNKS[0]],
                ).then_inc(s_dz, 16)
                for mb in range(2, MB):
                    gp.dma_start(
                        out=zoh[:, mb], in_=zoh_ext[:, mb]
                    ).then_inc(s_dz0, 16)
                c0 = CHUNKS[0]
                for c, NC in enumerate(CHUNKS):
                    if c == 0:
                        continue
                    if c >= 2:
                        gp.wait_ge(s_ec, 8 * (c - 1))
                    gp.dma_start(
                        out=zhb[c % 2][:, 0:MB * NC],
                        in_=zoht_ext[:, MB * c0:MB * (c0 + NC)],
                    ).then_inc(s_dz, 16)
                    c0 += NC
                gp.wait_ge(s_ln, NT - 1)
                gp.wait_ge(s_ec, NT - 1)
                gp.dma_start(
                    out=out_ext[:, 0:2 * (NT - 1)], in_=pt[:, 0:2 * (NT - 1)]
                ).then_inc(s_out, 16)
                gp.wait_ge(s_out, 16)

            @block.tensor
            def _(tensor):
                # warmup: ramp the PE clock to full p-state during the input
                # DMA window (results are garbage, bank is reset by the first
                # real start=True chain; PE is in-order so no sync needed)
                tensor.wait_ge(s_dz0, 16)
                zw = zoh[:, 0].rearrange("p k m -> p (k m)")
                for w in range(9):
                    tensor.matmul(
                        acc[NPSUM - 1][:, 0:504],
                        zw[:, 0:256].rearrange("p (t m) -> p t m", t=2),
                        zw[:, 0:1008].rearrange("p (t n) -> p t n", t=2),
                        start=True,
                        stop=True,
                        perf_mode=mybir.MatmulPerfMode.DoubleRow,
                    )
                for c, NC in enumerate(CHUNKS):
                    for mb in range(MB):
                        t = c * MB + mb
                        if c == 0:
                            tensor.wait_ge(s_dz0, 16 * (mb + 1))
                            if mb == 0:
                                tensor.wait_ge(s_dj2, 16)
                        if mb == 0:
                            tensor.wait_ge(s_dj, 16 * (c + 1))
                        if t >= NPSUM and t % 3 == 0:
                            # psum banks free for chains t..t+2: exp + ec of
                            # chain t-NPSUM+2 done (covers all three)
                            tensor.wait_ge(s_exp, t - NPSUM + 3)
                            tensor.wait_ge(s_ec, t - NPSUM + 3)
                        a = acc[t % NPSUM]
                        for kk in range(KP):
                            ins = tensor.matmul(
                                a[:, 0:NC],
                                zoh[:, mb, 2 * kk:2 * kk + 2, :],
                                jtb[c % 2][
                                    :, 2 * kk * NC:(2 * kk + 2) * NC
                                ].rearrange("p (t n) -> p t n", t=2),
                                start=(kk == 0),
                                stop=(kk == KP - 1),
                                perf_mode=mybir.MatmulPerfMode.DoubleRow,
                            )
                        ins.then_inc(s_mm)

            @block.scalar
            def _(scalar):
                # chunk-0 first half, in parallel with SP's second half
                scalar.dma_start(
                    out=jtb[0][:, 0:HALF0], in_=jt_ext[:, 0:HALF0]
                ).then_inc(s_dj2, 16)
                for c, NC in enumerate(CHUNKS):
                    G = NC // Q_AA
                    for mb in range(MB):
                        t = c * MB + mb
                        scalar.wait_ge(s_mm, t + 1)
                        if t == NT - 1:
                            # final chain: just stage E for the host
                            scalar.activation(
                                eL[:],
                                acc[t % NPSUM][:, 0:NC],
                                mybir.ActivationFunctionType.Copy,
                            ).then_inc(s_cp)
                            continue
                        if t >= NE:
                            scalar.wait_ge(s_red, t - NE + 1)
                        scalar.activation(
                            et[t % NE][:, 0:NC],
                            acc[t % NPSUM][:, 0:NC],
                            mybir.ActivationFunctionType.Exp,
                            scale=1.0 / SCALE,
                        ).then_inc(s_exp)
                        scalar.wait_ge(s_red, t + 1)
                        scalar.activation(
                            lse[:, 0:G],
                            se[t % 2][:, 0:G],
                            mybir.ActivationFunctionType.Ln,
                            accum_out=pt[:, 2 * t:2 * t + 1],
                        ).then_inc(s_ln)

            @block.vector
            def _(vector):
                for c, NC in enumerate(CHUNKS):
                    G = NC // Q_AA
                    for mb in range(MB):
                        t = c * MB + mb
                        if t == NT - 1:
                            continue    # final chain: host epilogue
                        vector.wait_ge(s_exp, t + 1)
                        if t >= 2:
                            vector.wait_ge(s_ln, t - 1)
                        vector.tensor_reduce(
                            se[t % 2][:, 0:G],
                            et[t % NE][:, 0:NC].rearrange(
                                "p (g q) -> p g q", q=Q_AA
                            ),
                            axis=mybir.AxisListType.X,
                            op=mybir.AluOpType.add,
                        ).then_inc(s_red)
                        if mb == 0:
                            vector.wait_ge(s_dz, 16 * (c + 1))
                        vector.scalar_tensor_tensor(
                            out=scr[:, 0:NC],
                            in0=acc[t % NPSUM][:, 0:NC],
                            scalar=1.0,
                            in1=zhb[c % 2][:, mb * NC:(mb + 1) * NC],
                            op0=mybir.AluOpType.mult,
                            op1=mybir.AluOpType.mult,
                            accum_out=pt[:, 2 * t + 1:2 * t + 2],
                        ).then_inc(s_ec)

    _CACHE["nc"] = nc
    return nc


def _softmax(x, axis):
    x = x - x.max(axis=axis, keepdims=True)
    e = np.exp(x)
    return e / e.sum(axis=axis, keepdims=True)


def _prologue(reps_matrix, Q, K, V_metric):
    """A, Vaa, and the scaled coupling tensor X4[i,j,q,a]; plus reg term."""
    scores = np.matmul(Q, K.transpose(0, 2, 1)) / np.sqrt(np.float32(DK))
    probs = _softmax(scores, -1)
    A = 0.5 * (probs + probs.transpose(0, 2, 1))           # (H, L, L)

    V1 = np.einsum("qd,hdv->hqv", reps_matrix, V_metric)   # (H, q, dv)
    gamma = 1.0 / V1.shape[1]
    sq = np.sum(V1 * V1, axis=-1)
    D2 = sq[:, :, None] + sq[:, None, :] - 2.0 * np.einsum("hqv,hav->hqa", V1, V1)
    Vaa = np.exp(-gamma * np.maximum(D2, 0.0))             # (H, q, q)

    # X4[i,j,q,a] = SCALE * sum_h A[h,i,j] Vaa[h,q,a], diagonal i==j zeroed
    X = A.reshape(H, L * L).T @ (Vaa * SCALE).reshape(H, Q_AA * Q_AA)
    X4 = X.reshape(L, L, Q_AA, Q_AA)
    X4[np.arange(L), np.arange(L)] = 0.0
    Xf = X4.reshape(-1)
    reg = LAMBDA * float(np.dot(Xf, Xf)) / (SCALE * SCALE)
    return X4, reg


def kernel(reps_matrix, Q, K, V_metric, Z, weights):
    global LAST_RESULTS, DEVICE_NS
    reps_matrix = np.asarray(reps_matrix, np.float32)
    Q = np.asarray(Q, np.float32)
    K = np.asarray(K, np.float32)
    V_metric = np.asarray(V_metric, np.float32)
    Zi = np.asarray(Z).astype(np.int64)
    weights = np.asarray(weights, np.float32)

    X4, reg = _prologue(reps_matrix, Q, K, V_metric)

    # Safety bound for the shift-free on-device logsumexp:
    # max_{q,r,m} E <= max_{q,r} sum_j max_a J[r,j,q,a]
    emax = float(X4.max(axis=3).sum(axis=1).max()) / SCALE
    colidx = np.arange(L)[:, None] * Q_AA + Zi             # (L, M)

    s_all = None
    if emax < 80.0:
        try:
            # Jmat[(i,q),(j,a)] is symmetric; device K-layout [kp, kb, f]
            Jmat = X4.transpose(0, 2, 1, 3).reshape(F, F)
            jt8 = Jmat.reshape(KB, 128, F).astype(FP8)
            jt_np = jt8.transpose(1, 0, 2)          # (128, KB, F) view
            # chunk-major pre-tile: per partition, chunk c is one
            # contiguous (KB*Nc) block
            bounds = np.cumsum([0] + CHUNKS)
            jt_np = np.concatenate(
                [np.ascontiguousarray(jt_np[:, :, a:b]).reshape(128, -1)
                 for a, b in zip(bounds[:-1], bounds[1:])], axis=1
            )

            in_maps = []
            for c in range(N_CORES):
                ci = colidx[:, c * M_LOC:(c + 1) * M_LOC]
                zfull = np.zeros((F, M_LOC), FP8)
                zfull[ci, np.arange(M_LOC)[None, :]] = 1.0
                # [kp, mb, kb, mcol] so each mb-block is one contiguous DMA
                zoh_np = np.ascontiguousarray(
                    zfull.reshape(KB, 128, MB, 128).transpose(1, 2, 0, 3)
                )
                zt = np.zeros((M_LOC, F), FP8)
                zt[np.arange(M_LOC)[:, None], ci.T] = 1.0
                zoht_np = zt.reshape(MB, 128, F).transpose(1, 0, 2)
                zoht_np = np.concatenate(
                    [np.ascontiguousarray(zoht_np[:, :, a:b]).reshape(128, -1)
                     for a, b in zip(bounds[:-1], bounds[1:])], axis=1
                )
                in_maps.append({"jt": jt_np, "zoh": zoh_np, "zoht": zoht_np})

            nc = _build_graph()
            import time as _time
            _t0 = _time.time()
            res = run_bass_kernel_spmd(nc, in_maps, list(range(N_CORES)))
            DEVICE_NS = int((_time.time() - _t0) * 1e9)
            LAST_RESULTS = res
            parts = []
            NRL = CHUNKS[-1] // Q_AA                  # r-groups in final chain
            for c in range(N_CORES):
                o = np.array(np.asarray(res.results[c]["out"], np.float32))
                o = o.reshape(128, NCH, MB, 2)        # slot = (c*MB+mb)*2+k
                o[:, NCH - 1, MB - 1, :] = 0.0        # final chain: host path
                sl = o[..., 0].sum(axis=1)            # (128, MB) sum_r lse
                ec = o[..., 1].sum(axis=1) / SCALE    # (128, MB) sum_r Ec
                s_core = (ec - sl).T.reshape(M_LOC).copy()  # m = mb*128+p
                # final chain (last NRL r-groups x last m-block) from raw E
                eL = np.asarray(res.results[c]["oute"], np.float32) / SCALE
                E3 = eL.reshape(128, NRL, Q_AA)       # [p, r_local, q]
                mx = E3.max(axis=2)
                lge = mx + np.log(np.exp(E3 - mx[..., None]).sum(axis=2))
                zsel = Zi[L - NRL:, c * M_LOC + (MB - 1) * 128:
                          c * M_LOC + MB * 128]       # (NRL, 128)
                ecv = np.take_along_axis(
                    E3, zsel.T[:, :, None], axis=2
                )[..., 0]                             # (128, NRL)
                s_core[(MB - 1) * 128:] += (ecv - lge).sum(axis=1)
                parts.append(s_core)
            s_all = np.concatenate(parts)             # (M,)
        except Exception:
            import traceback
            print("kernel: device path failed, CPU fallback:", file=sys.stderr)
            traceback.print_exc()
            s_all = None

    if s_all is None:
        # host fallback: same contraction in fp32 on CPU
        Jmat = X4.transpose(0, 2, 1, 3).reshape(F, F) / SCALE
        parts = []
        for c in range(N_CORES):
            ci = colidx[:, c * M_LOC:(c + 1) * M_LOC]
            zfull = np.zeros((F, M_LOC), np.float32)
            zfull[ci, np.arange(M_LOC)[None, :]] = 1.0
            E = (Jmat @ zfull).reshape(L, Q_AA, M_LOC)
            mx = E.max(axis=1)
            lge = mx + np.log(np.sum(np.exp(E - mx[:, None, :]), axis=1))
            loc = Zi[:, c * M_LOC:(c + 1) * M_LOC]
            Ec = np.take_along_axis(E, loc[:, None, :], axis=1)[:, 0, :]
            parts.append(np.sum(Ec - lge, axis=0))
        s_all = np.concatenate(parts)

    pl = -float(np.dot(weights, s_all))
    return np.float32(pl + reg)


# revision 57
# speedup vs baseline: 1.0207x; 1.0148x over previous
"""AttentionDCA pseudo-likelihood loss on 8 Trainium2 NeuronCores.

Data-parallel over the MSA axis M (1024 sequences per core).

Host (cheap): attention map A, RBF kernel Vaa, coupling matrix
Jmat[(r,q),(j,a)] = sum_h A[h,r,j] Vaa[h,q,a] with the r==j diagonal
blocks zeroed.  Jmat is symmetric (A and Vaa are both symmetric), so the
same buffer serves as the matmul rhs without a transpose.

Device (dominant): E^T[m, f] = sum_k Zoh[k, m] * Jmat[k, f] as fp8-E4M3
matmuls in DoubleRow perf mode (K=256 per instruction), f tiled in
chunks of 504 = 24 complete 21-wide q-groups.  The epilogue is fused on
device: per chunk, exp -> 21-group sums -> Ln (with row accumulation)
gives sum_r log-sum-exp_q E, and scalar_tensor_tensor against the
transposed one-hot gives sum_r E[Z[r,m]].  Only 2 partial scalars per
(m, chunk) leave the device (90 KB/core instead of 22 MB).

The graph is raw Bass blocks (explicit per-engine streams + semaphores):
the TileContext scheduler emits multi-wait instructions this container's
walrus build rejects ("Too many sync wait commands").

J is pre-scaled by SCALE (folded into Vaa); the Exp activation's scale
argument un-scales it, and the host divides the Ec partials by SCALE.
E >= 0 and max E ~ 4 here, so the logsumexp needs no max-shift (guarded
by an upper bound computed on the host; falls back to CPU if violated).
"""

import sys
import numpy as np

for p in ("/opt/trn_rl_repo", "/root/.axon_site/_ro/trn_rl_repo"):
    if p not in sys.path:
        sys.path.insert(0, p)

import ml_dtypes

import concourse.bass as bass
from concourse import mybir
from concourse.bass_utils import run_bass_kernel_spmd

Q_AA = 21
H = 32
L = 256
DK = 32
M_TOT = 8192
N_CORES = 8
M_LOC = M_TOT // N_CORES          # 1024
MB = M_LOC // 128                 # 8 output-row blocks (m)
F = L * Q_AA                      # 5376 flattened (pos, aa) dim
KB = F // 128                     # 42 K-blocks of 128
KP = KB // 2                      # 21 DoubleRow pairs
CHUNKS = [504] * 10 + [336]           # f-chunks, each a whole number of
                                      # 21-wide q-groups (psum bank caps at 504)
NCH = len(CHUNKS)
NT = NCH * MB                     # 88 matmul chains
LAMBDA = 1e-3
SCALE = 256.0                     # J pre-scale for fp8 range use
FP8 = ml_dtypes.float8_e4m3fn
BF16 = ml_dtypes.bfloat16

NPSUM = 6                         # psum banks in rotation
NE = 3                            # exp-result buffers

_CACHE = {}
LAST_RESULTS = None               # for test harness introspection
DEVICE_NS = None                  # wall-clock of the device submit+run+fetch


def _build_graph():
    if "nc" in _CACHE:
        return _CACHE["nc"]
    nc = bass.Bass()
    f8 = mybir.dt.float8e4
    bf = mybir.dt.bfloat16
    f32 = mybir.dt.float32
    # jt / zoht are chunk-major pre-tiled on the host so every chunk DMA is
    # one contiguous run per partition (sub-512B segments cost 2x DMA time);
    # zoh is mb-major so PE can start after the first 1/8th arrives
    jt_ext = nc.declare_dram_parameter("jt", [128, KB * F], f8, isOutput=False)
    zoh_ext = nc.declare_dram_parameter("zoh", [128, MB, KB, 128], f8, isOutput=False)
    zoht_ext = nc.declare_dram_parameter("zoht", [128, MB * F], f8, isOutput=False)
    out_ext = nc.declare_dram_parameter("out", [128, NT * 2], f32, isOutput=True)
    # raw E block of the final chain — its lse/Ec run on the host so the
    # device tail is a Copy + DMA on idle queues instead of Exp->reduce->Ln
    oute_ext = nc.declare_dram_parameter(
        "oute", [128, CHUNKS[-1]], f32, isOutput=True
    )

    from contextlib import ExitStack
    with ExitStack() as ctx:
        def sem(name):
            return ctx.enter_context(nc.semaphore(name))

        def sb(name, shape, dtype):
            return ctx.enter_context(nc.sbuf_tensor(name, shape, dtype))

        s_dz0 = sem("s_dz0")      # zoh loaded
        s_dj = sem("s_dj")        # jt chunks loaded (SP queue)
        s_dj2 = sem("s_dj2")      # jt chunk-0 first half (ACT queue)
        s_dz = sem("s_dz")        # zht chunks loaded
        s_mm = sem("s_mm")        # matmul chains done
        s_exp = sem("s_exp")      # exp done
        s_ln = sem("s_ln")        # ln done
        s_red = sem("s_red")      # group-sum done
        s_ec = sem("s_ec")        # ec stt done
        s_cp = sem("s_cp")        # final-chain E copied to sbuf
        s_wz = sem("s_wz")        # warmup operand tile zeroed
        s_out = sem("s_out")      # partials stored (Pool queue)
        s_oute = sem("s_oute")    # raw E block stored (SP queue)

        zoh = sb("sb_zoh", [128, MB, KB, 128], f8)
        # flat chunk buffers: every chunk lands contiguous regardless of size
        jtb = [sb(f"jtb{i}", [128, KB * 504], f8) for i in range(2)]
        zhb = [sb(f"zhb{i}", [128, MB * 504], f8) for i in range(2)]
        et = [sb(f"et{i}", [128, 504], f32) for i in range(NE)]
        se = [sb(f"se{i}", [128, 24], f32) for i in range(2)]
        lse = sb("lse", [128, 24], f32)
        scr = sb("scr", [128, 504], f32)
        eL = sb("eL", [128, CHUNKS[-1]], f32)
        pt = sb("pt", [128, NT * 2], f32)
        acc = [
            ctx.enter_context(nc.psum_tensor(f"acc{i}", [128, 504], f32))
            for i in range(NPSUM)
        ]

        with nc.Block() as block:

            HALF0 = (KB // 2) * CHUNKS[0]   # chunk-0 ACT/SP split (kb 0..20)
            Q1 = 11 * CHUNKS[0]             # ACT piece split (kb 0..10 | 11..20)
            Q2 = HALF0                      # SP pieces start (kb 21)
            Q3 = 32 * CHUNKS[0]             # SP piece split (kb 21..31 | 32..41)

            @block.sync
            def _(sync):
                # SP queue: jt chunk stream (chunk 0 second half only — the
                # first half arrives in parallel on the ACT queue so PE can
                # start ~4us earlier)
                c0 = 0
                for c, NC in enumerate(CHUNKS):
                    if c == 0:
                        # chunk 0 in two pieces: the first lands sooner and
                        # the second's DMA init hides behind the first's
                        # transfer, so chain 0 can start mid-load
                        for lo, hi in ((Q2, Q3), (Q3, KB * NC)):
                            sync.dma_start(
                                out=jtb[0][:, lo:hi],
                                in_=jt_ext[:, lo:hi],
                            ).then_inc(s_dj, 16)
                        c0 += NC
                        continue
                    if c >= 2:
                        # buffer c%2 readers: all chains of chunk c-2
                        sync.wait_ge(s_mm, 8 * (c - 1))
                    sync.dma_start(
                        out=jtb[c % 2][:, 0:KB * NC],
                        in_=jt_ext[:, KB * c0:KB * (c0 + NC)],
                    ).then_inc(s_dj, 16)
                    c0 += NC
                sync.wait_ge(s_cp, 1)
                sync.dma_start(out=oute_ext[:], in_=eL[:]).then_inc(s_oute, 16)
                sync.wait_ge(s_oute, 16)

            @block.gpsimd
            def _(gp):
                # Pool queue: one-hots in, partials out — overlaps the SP jt
                # stream.  zht c0 is needed by the DVE Ec path right after
                # the first chain, so it goes out after just two zoh blocks.
                for mb in range(2):
                    gp.dma_start(
                        out=zoh[:, mb], in_=zoh_ext[:, mb]
                    ).then_inc(s_dz0, 16)
                gp.dma_start(
                    out=zhb[0][:, 0:MB * CHUNKS[0]],
                    in_=zoht_ext[:, 0:MB * CHUNKS[0]],
                ).then_inc(s_dz, 16)
                for mb in range(2, MB):
                    gp.dma_start(
                        out=zoh[:, mb], in_=zoh_ext[:, mb]
                    ).then_inc(s_dz0, 16)
                c0 = CHUNKS[0]
                for c, NC in enumerate(CHUNKS):
                    if c == 0:
                        continue
                    if c >= 2:
                        gp.wait_ge(s_ec, 8 * (c - 1))
                    gp.dma_start(
                        out=zhb[c % 2][:, 0:MB * NC],
                        in_=zoht_ext[:, MB * c0:MB * (c0 + NC)],
                    ).then_inc(s_dz, 16)
                    c0 += NC
                gp.wait_ge(s_ln, NT - 1)
                gp.wait_ge(s_ec, NT - 1)
                gp.dma_start(
                    out=out_ext[:, 0:2 * (NT - 1)], in_=pt[:, 0:2 * (NT - 1)]
                ).then_inc(s_out, 16)
                gp.wait_ge(s_out, 16)

            @block.tensor
            def _(tensor):
                # warmup: ramp the PE clock to full p-state during the input
                # DMA window (results are garbage, bank is reset by the first
                # real start=True chain; PE is in-order so no sync needed).
                # Gated on a DVE memset (~0.8us) instead of any DMA so the
                # ramp (3us) completes before the first real chain.
                tensor.wait_ge(s_wz, 1)
                zw = eL[:].bitcast(mybir.dt.float8e4)
                for w in range(30):
                    tensor.matmul(
                        acc[NPSUM - 1][:, 0:504],
                        zw[:, 0:256].rearrange("p (t m) -> p t m", t=2),
                        zw[:, 0:1008].rearrange("p (t n) -> p t n", t=2),
                        start=True,
                        stop=True,
                        perf_mode=mybir.MatmulPerfMode.DoubleRow,
                    )
                for c, NC in enumerate(CHUNKS):
                    for mb in range(MB):
                        t = c * MB + mb
                        if c == 0:
                            tensor.wait_ge(s_dz0, 16 * (mb + 1))
                        if mb == 0 and c >= 1:
                            tensor.wait_ge(s_dj, 16 * (c + 2))
                        if t >= NPSUM and t % 3 == 0:
                            # psum banks free for chains t..t+2: exp + ec of
                            # chain t-NPSUM+2 done (covers all three)
                            tensor.wait_ge(s_exp, t - NPSUM + 3)
                            tensor.wait_ge(s_ec, t - NPSUM + 3)
                        a = acc[t % NPSUM]
                        for kk in range(KP):
                            if t == 0:
                                # jt chunk-0 pieces: kb0-10 | kb11-20 (ACT),
                                # kb21-31 | kb32-41 (SP), consumed in order
                                if kk == 0:
                                    tensor.wait_ge(s_dj2, 16)
                                elif kk == 5:
                                    tensor.wait_ge(s_dj2, 32)
                                elif kk == 10:
                                    tensor.wait_ge(s_dj, 16)
                                elif kk == 16:
                                    tensor.wait_ge(s_dj, 32)
                            ins = tensor.matmul(
                                a[:, 0:NC],
                                zoh[:, mb, 2 * kk:2 * kk + 2, :],
                                jtb[c % 2][
                                    :, 2 * kk * NC:(2 * kk + 2) * NC
                                ].rearrange("p (t n) -> p t n", t=2),
                                start=(kk == 0),
                                stop=(kk == KP - 1),
                                perf_mode=mybir.MatmulPerfMode.DoubleRow,
                            )
                        ins.then_inc(s_mm)

            @block.scalar
            def _(scalar):
                # chunk-0 first half in two pieces, parallel with SP's half
                scalar.dma_start(
                    out=jtb[0][:, 0:Q1], in_=jt_ext[:, 0:Q1]
                ).then_inc(s_dj2, 16)
                scalar.dma_start(
                    out=jtb[0][:, Q1:HALF0], in_=jt_ext[:, Q1:HALF0]
                ).then_inc(s_dj2, 16)
                for c, NC in enumerate(CHUNKS):
                    G = NC // Q_AA
                    for mb in range(MB):
                        t = c * MB + mb
                        scalar.wait_ge(s_mm, t + 1)
                        if t == NT - 1:
                            # final chain: just stage E for the host
                            scalar.activation(
                                eL[:],
                                acc[t % NPSUM][:, 0:NC],
                                mybir.ActivationFunctionType.Copy,
                            ).then_inc(s_cp)
                            continue
                        if t >= NE:
                            scalar.wait_ge(s_red, t - NE + 1)
                        scalar.activation(
                            et[t % NE][:, 0:NC],
                            acc[t % NPSUM][:, 0:NC],
                            mybir.ActivationFunctionType.Exp,
                            scale=1.0 / SCALE,
                        ).then_inc(s_exp)
                        scalar.wait_ge(s_red, t + 1)
                        scalar.activation(
                            lse[:, 0:G],
                            se[t % 2][:, 0:G],
                            mybir.ActivationFunctionType.Ln,
                            accum_out=pt[:, 2 * t:2 * t + 1],
                        ).then_inc(s_ln)

            @block.vector
            def _(vector):
                vector.memset(eL[:], 0).then_inc(s_wz)
                for c, NC in enumerate(CHUNKS):
                    G = NC // Q_AA
                    for mb in range(MB):
                        t = c * MB + mb
                        if t == NT - 1:
                            continue    # final chain: host epilogue
                        vector.wait_ge(s_exp, t + 1)
                        if t >= 2:
                            vector.wait_ge(s_ln, t - 1)
                        vector.tensor_reduce(
                            se[t % 2][:, 0:G],
                            et[t % NE][:, 0:NC].rearrange(
                                "p (g q) -> p g q", q=Q_AA
                            ),
                            axis=mybir.AxisListType.X,
                            op=mybir.AluOpType.add,
                        ).then_inc(s_red)
                        if mb == 0:
                            vector.wait_ge(s_dz, 16 * (c + 1))
                        vector.scalar_tensor_tensor(
                            out=scr[:, 0:NC],
                            in0=acc[t % NPSUM][:, 0:NC],
                            scalar=1.0,
                            in1=zhb[c % 2][:, mb * NC:(mb + 1) * NC],
                            op0=mybir.AluOpType.mult,
                            op1=mybir.AluOpType.mult,
                            accum_out=pt[:, 2 * t + 1:2 * t + 2],
                        ).then_inc(s_ec)

    _CACHE["nc"] = nc
    return nc


def _softmax(x, axis):
    x = x - x.max(axis=axis, keepdims=True)
    e = np.exp(x)
    return e / e.sum(axis=axis, keepdims=True)


def _prologue(reps_matrix, Q, K, V_metric):
    """A, Vaa, and the scaled coupling tensor X4[i,j,q,a]; plus reg term."""
    scores = np.matmul(Q, K.transpose(0, 2, 1)) / np.sqrt(np.float32(DK))
    probs = _softmax(scores, -1)
    A = 0.5 * (probs + probs.transpose(0, 2, 1))           # (H, L, L)

    V1 = np.einsum("qd,hdv->hqv", reps_matrix, V_metric)   # (H, q, dv)
    gamma = 1.0 / V1.shape[1]
    sq = np.sum(V1 * V1, axis=-1)
    D2 = sq[:, :, None] + sq[:, None, :] - 2.0 * np.einsum("hqv,hav->hqa", V1, V1)
    Vaa = np.exp(-gamma * np.maximum(D2, 0.0))             # (H, q, q)

    # X4[i,j,q,a] = SCALE * sum_h A[h,i,j] Vaa[h,q,a], diagonal i==j zeroed
    X = A.reshape(H, L * L).T @ (Vaa * SCALE).reshape(H, Q_AA * Q_AA)
    X4 = X.reshape(L, L, Q_AA, Q_AA)
    X4[np.arange(L), np.arange(L)] = 0.0
    Xf = X4.reshape(-1)
    reg = LAMBDA * float(np.dot(Xf, Xf)) / (SCALE * SCALE)
    return X4, reg


def kernel(reps_matrix, Q, K, V_metric, Z, weights):
    global LAST_RESULTS, DEVICE_NS
    reps_matrix = np.asarray(reps_matrix, np.float32)
    Q = np.asarray(Q, np.float32)
    K = np.asarray(K, np.float32)
    V_metric = np.asarray(V_metric, np.float32)
    Zi = np.asarray(Z).astype(np.int64)
    weights = np.asarray(weights, np.float32)

    X4, reg = _prologue(reps_matrix, Q, K, V_metric)

    # Safety bound for the shift-free on-device logsumexp:
    # max_{q,r,m} E <= max_{q,r} sum_j max_a J[r,j,q,a]
    emax = float(X4.max(axis=3).sum(axis=1).max()) / SCALE
    colidx = np.arange(L)[:, None] * Q_AA + Zi             # (L, M)

    s_all = None
    if emax < 80.0:
        try:
            # Jmat[(i,q),(j,a)] is symmetric; device K-layout [kp, kb, f]
            Jmat = X4.transpose(0, 2, 1, 3).reshape(F, F)
            jt8 = Jmat.reshape(KB, 128, F).astype(FP8)
            jt_np = jt8.transpose(1, 0, 2)          # (128, KB, F) view
            # chunk-major pre-tile: per partition, chunk c is one
            # contiguous (KB*Nc) block
            bounds = np.cumsum([0] + CHUNKS)
            jt_np = np.concatenate(
                [np.ascontiguousarray(jt_np[:, :, a:b]).reshape(128, -1)
                 for a, b in zip(bounds[:-1], bounds[1:])], axis=1
            )

            in_maps = []
            for c in range(N_CORES):
                ci = colidx[:, c * M_LOC:(c + 1) * M_LOC]
                zfull = np.zeros((F, M_LOC), FP8)
                zfull[ci, np.arange(M_LOC)[None, :]] = 1.0
                # [kp, mb, kb, mcol] so each mb-block is one contiguous DMA
                zoh_np = np.ascontiguousarray(
                    zfull.reshape(KB, 128, MB, 128).transpose(1, 2, 0, 3)
                )
                zt = np.zeros((M_LOC, F), FP8)
                zt[np.arange(M_LOC)[:, None], ci.T] = 1.0
                zoht_np = zt.reshape(MB, 128, F).transpose(1, 0, 2)
                zoht_np = np.concatenate(
                    [np.ascontiguousarray(zoht_np[:, :, a:b]).reshape(128, -1)
                     for a, b in zip(bounds[:-1], bounds[1:])], axis=1
                )
                in_maps.append({"jt": jt_np, "zoh": zoh_np, "zoht": zoht_np})

            nc = _build_graph()
            import time as _time
            _t0 = _time.time()
            res = run_bass_kernel_spmd(nc, in_maps, list(range(N_CORES)))
            DEVICE_NS = int((_time.time() - _t0) * 1e9)
            LAST_RESULTS = res
            parts = []
            NRL = CHUNKS[-1] // Q_AA                  # r-groups in final chain
            for c in range(N_CORES):
                o = np.array(np.asarray(res.results[c]["out"], np.float32))
                o = o.reshape(128, NCH, MB, 2)        # slot = (c*MB+mb)*2+k
                o[:, NCH - 1, MB - 1, :] = 0.0        # final chain: host path
                sl = o[..., 0].sum(axis=1)            # (128, MB) sum_r lse
                ec = o[..., 1].sum(axis=1) / SCALE    # (128, MB) sum_r Ec
                s_core = (ec - sl).T.reshape(M_LOC).copy()  # m = mb*128+p
                # final chain (last NRL r-groups x last m-block) from raw E
                eL = np.asarray(res.results[c]["oute"], np.float32) / SCALE
                E3 = eL.reshape(128, NRL, Q_AA)       # [p, r_local, q]
                mx = E3.max(axis=2)
                lge = mx + np.log(np.exp(E3 - mx[..., None]).sum(axis=2))
                zsel = Zi[L - NRL:, c * M_LOC + (MB - 1) * 128:
                          c * M_LOC + MB * 128]       # (NRL, 128)
                ecv = np.take_along_axis(
                    E3, zsel.T[:, :, None], axis=2
                )[..., 0]                             # (128, NRL)
                s_core[(MB - 1) * 128:] += (ecv - lge).sum(axis=1)
                parts.append(s_core)
            s_all = np.concatenate(parts)             # (M,)
        except Exception:
            import traceback
            print("kernel: device path failed, CPU fallback:", file=sys.stderr)
            traceback.print_exc()
            s_all = None

    if s_all is None:
        # host fallback: same contraction in fp32 on CPU
        Jmat = X4.transpose(0, 2, 1, 3).reshape(F, F) / SCALE
        parts = []
        for c in range(N_CORES):
            ci = colidx[:, c * M_LOC:(c + 1) * M_LOC]
            zfull = np.zeros((F, M_LOC), np.float32)
            zfull[ci, np.arange(M_LOC)[None, :]] = 1.0
            E = (Jmat @ zfull).reshape(L, Q_AA, M_LOC)
            mx = E.max(axis=1)
            lge = mx + np.log(np.sum(np.exp(E - mx[:, None, :]), axis=1))
            loc = Zi[:, c * M_LOC:(c + 1) * M_LOC]
            Ec = np.take_along_axis(E, loc[:, None, :], axis=1)[:, 0, :]
            parts.append(np.sum(Ec - lge, axis=0))
        s_all = np.concatenate(parts)

    pl = -float(np.dot(weights, s_all))
    return np.float32(pl + reg)
